# revision 2
# baseline (speedup 1.0000x reference)
"""CKAN (gnn_message_passing) Trainium2 kernel.

Data-parallel over 8 NeuronCores (512 batch rows each), no collectives.
Entity-embedding rows are fetched in two levels to fit dma_gather's int16
index limit:
  phase A: sorted-unique h∪t rows, gathered from the full table with
           windowed calls (idx - 32768*w fits int16; sorted windows give
           contiguous compact dests), staged back to HBM;
  phase B: per-(b,t) rows gathered from the staging table with local int16
           indices.
The per-column MLP runs feature-major (PE transposes of gathered rows).
Relation embeddings enter pass 1 as a one-hot matmul against a
host-premultiplied [32, 128] table.  gate2 and att3 run as "flip" matmuls
(activations as lhsT) producing batch-major [g2 | s3], so the softmax over
neighbors and the weighted sum run on natural [128b, t] tiles.
Columns are processed in t-major order (col = t*512 + b).
"""
import sys
sys.path.insert(0, '/opt/trn_rl_repo')
import numpy as np

# ---- problem dims (overridable for small-scale sim tests) ----
DIM = 64
N_ENTITY = 100000
N_RELATION = 32
N_LAYER = 2
B = 4096
T = 32
N_CORES = 8
WIN = 32768
NW_CAP = None   # computed in _dims()
_NC_CACHE = None


def _dims():
    b_core = B // N_CORES
    n_col = b_core * T
    nwin = (N_ENTITY + WIN - 1) // WIN
    if NW_CAP is not None:
        caps = list(NW_CAP)
    else:
        # expected uniques per full window for 2*n_col draws over N_ENTITY,
        # with ~15 sigma headroom, rounded to 256
        import math
        caps = []
        for w in range(nwin):
            width = min(WIN, N_ENTITY - WIN * w)
            mean = width * (1.0 - (1.0 - 1.0 / N_ENTITY) ** (2 * n_col))
            cap = int(mean + 8 * math.sqrt(max(mean, 1.0)) + 64)
            caps.append(-(-cap // 128) * 128)
    na = sum(caps)
    assert na % 128 == 0
    return b_core, n_col, nwin, caps, na


def _wrap_idx16(a):
    """int16 vector -> dma_gather idx layout [128, ceil(n/16)]."""
    a = np.asarray(a, dtype=np.int16)
    n = len(a)
    pad = (-n) % 16
    if pad:
        a = np.concatenate([a, np.full(pad, -1, np.int16)])
    w = a.reshape(-1, 16).T.copy()
    return np.tile(w, (8, 1))


def _host_prep_tl(h_flat, t_flat):
    b_core, n_col, nwin, caps, na = _dims()
    uni = np.unique(np.concatenate([h_flat, t_flat]))
    val_to_pos = np.full(N_ENTITY, -1, np.int32)
    idxA_parts = []
    off = 0
    for w in range(nwin):
        lo, hi = WIN * w, min(WIN * (w + 1), N_ENTITY)
        seg = uni[(uni >= lo) & (uni < hi)]
        cap = caps[w]
        assert len(seg) <= cap, f"window {w} overflow: {len(seg)} > {cap}"
        val_to_pos[seg] = off + np.arange(len(seg), dtype=np.int32)
        assert len(seg) == 0 or off + len(seg) - 1 <= 32767, "position overflow"
        fill = (seg[-1] - lo) if len(seg) else 0
        seg_l = np.concatenate([(seg - lo).astype(np.int16),
                                np.full(cap - len(seg), fill, np.int16)])
        idxA_parts.append(seg_l)
        off += cap
    idxA = np.concatenate(idxA_parts)
    h_loc = val_to_pos[h_flat]
    t_loc = val_to_pos[t_flat]
    assert (h_loc >= 0).all() and (t_loc >= 0).all()
    return idxA, h_loc.astype(np.int16), t_loc.astype(np.int16)


TL_LIST = [("u", 0), ("u", 1), ("i", 0), ("i", 1)]


def _build_program():
    import concourse.bacc as bacc
    import concourse.tile as tile
    from concourse import mybir
    from concourse.masks import make_identity
    from concourse import tile_sem_assignment as tsa

    # Tile assigns DMASW sem lanes round-robin in scheduled order, but each
    # lane is hardware-locked to SWDGE queue (lane % 4).  Force gather
    # instructions onto lanes consistent with their queue_num.
    if not getattr(tsa.TileClockTick, "_gather_lane_patched", False):
        _orig_assign_tick = tsa.TileClockTick._assign_tick

        def _patched_assign_tick(self, inst):
            if isinstance(inst, mybir.InstDMAGatherAnt):
                q = inst.queue_num
                tog = getattr(self, "_gather_lane_toggle", None)
                if tog is None:
                    tog = self._gather_lane_toggle = {}
                k = tog.get(q, 0)
                tog[q] = k ^ 1
                saved = self.next_sw_dma_idx
                self.next_sw_dma_idx = q + 4 * k
                try:
                    return _orig_assign_tick(self, inst)
                finally:
                    self.next_sw_dma_idx = saved
            return _orig_assign_tick(self, inst)

        tsa.TileClockTick._assign_tick = _patched_assign_tick
        tsa.TileClockTick._gather_lane_patched = True

    f32 = mybir.dt.float32
    f16 = mybir.dt.float16
    i16 = mybir.dt.int16
    AF = mybir.ActivationFunctionType
    ALU = mybir.AluOpType
    AX = mybir.AxisListType

    b_core, n_col, nwin, caps, na = _dims()
    NB = b_core // 128               # b-chunks (4)
    NH = 2                           # phase-B halves
    TH = T // NH                     # t per half (16)
    COLS_H = 128 * NB * TH           # columns per half (8192)
    GQ = COLS_H // 2                 # columns per B-gather call (4096)
    na_slots = na // 128

    nc = bacc.Bacc("TRN2", target_bir_lowering=False, debug=True,
                   num_swdge_queues=4)
    _qctr = [0]

    def _nextq():
        q = _qctr[0] % 4
        _qctr[0] += 1
        return q

    ent = nc.dram_tensor("ent", [N_ENTITY, DIM], f32, kind="ExternalInput")
    w1h = nc.dram_tensor("w1h", [64, 128], f16, kind="ExternalInput")
    r1p = nc.dram_tensor("r1p", [N_RELATION, 128], f16, kind="ExternalInput")
    w2 = nc.dram_tensor("w2", [64, 64], f16, kind="ExternalInput")
    wflip = nc.dram_tensor("wflip", [128, 65], f16, kind="ExternalInput")
    iota = nc.dram_tensor("iota", [N_RELATION, 1], f32, kind="ExternalInput")
    items16 = nc.dram_tensor("items16", [128, max(b_core // 16, 1)], i16,
                             kind="ExternalInput")
    idxA, idxh, idxt, rflat = {}, {}, {}, {}
    for k in range(4):
        idxA[k] = nc.dram_tensor(f"idxA{k}", [128, na // 16], i16, kind="ExternalInput")
        idxh[k] = nc.dram_tensor(f"idxh{k}", [128, n_col // 16], i16, kind="ExternalInput")
        idxt[k] = nc.dram_tensor(f"idxt{k}", [128, n_col // 16], i16, kind="ExternalInput")
        rflat[k] = nc.dram_tensor(f"rflat{k}", [1, n_col], f16, kind="ExternalInput")
    scores_hbm = nc.dram_tensor("scores", [b_core], f32, kind="ExternalOutput")

    with tile.TileContext(nc) as tc:
        with (
            tc.tile_pool(name="dram", bufs=1, space="DRAM") as dp,
            tc.tile_pool(name="const", bufs=1) as cp,
            tc.tile_pool(name="big", bufs=1) as bigp,
            tc.tile_pool(name="bg", bufs=2) as bgp,
            tc.tile_pool(name="work", bufs=2) as wp,
            tc.tile_pool(name="idx", bufs=2) as ixp,
            tc.tile_pool(name="keep", bufs=1) as kp,
            tc.tile_pool(name="psA", bufs=1, space="PSUM") as psA,
            tc.tile_pool(name="psB", bufs=2, space="PSUM") as psB,
            tc.tile_pool(name="psF", bufs=2, space="PSUM") as psF,
        ):
            stage = [dp.tile([na, 2 * DIM], f16, name=f"stage{k}") for k in range(4)]

            ident = cp.tile([128, 128], f32)
            make_identity(nc, ident[:])
            ident16 = cp.tile([128, 128], f16)
            nc.vector.tensor_copy(out=ident16[:], in_=ident[:])
            w1h_sb = cp.tile([64, 128], f16)
            nc.sync.dma_start(out=w1h_sb[:], in_=w1h[:])
            r1p_sb = cp.tile([N_RELATION, 128], f16)
            nc.sync.dma_start(out=r1p_sb[:], in_=r1p[:])
            w2_sb = cp.tile([64, 64], f16)
            nc.sync.dma_start(out=w2_sb[:], in_=w2[:])
            wflip_sb = cp.tile([128, 65], f16)
            nc.sync.dma_start(out=wflip_sb[:], in_=wflip[:])
            iota_sb = cp.tile([N_RELATION, 1], f32)
            nc.sync.dma_start(out=iota_sb[:], in_=iota[:])
            ones1 = cp.tile([1, N_RELATION], f16)
            nc.gpsimd.memset(ones1[:], 1.0)

            items_sb = cp.tile([128, max(b_core // 16, 1)], i16)
            nc.sync.dma_start(out=items_sb[:], in_=items16[:])
            iorig = kp.tile([128, NB, DIM], f32)
            nc.gpsimd.dma_gather(
                out_ap=iorig[:], in_ap=ent[:], idxs_ap=items_sb[:],
                num_idxs=b_core, num_idxs_reg=b_core, elem_size=DIM,
                queue_num=_nextq(), single_packet=False)

            otl = {}            # (k, q) -> [128, 64] layer outputs
            uorig_bm = [None] * NB

            for k, (tw, l) in enumerate(TL_LIST):
                # ---- phase A ----
                ia = ixp.tile([128, na // 16], i16, name=f"ia{k}", tag="ia")
                nc.sync.dma_start(out=ia[:], in_=idxA[k][:])
                # flat list of (window, offset, count) calls, <=2560 idx each
                acalls = []
                off = 0
                for w in range(nwin):
                    nw_ = caps[w]
                    nsplit = max(1, -(-nw_ // 3968))
                    step = -(-(-(-nw_ // nsplit)) // 128) * 128
                    done = 0
                    while done < nw_:
                        nn = min(step, nw_ - done)
                        acalls.append((w, off + done, nn))
                        done += nn
                    off += nw_
                # split calls into NPIECE pieces at call boundaries
                NPIECE = 3
                bounds = [0]
                for pi in range(1, NPIECE):
                    tgt = na * pi // NPIECE
                    bb = 0
                    for (_, o2, nn) in acalls:
                        if o2 >= tgt:
                            break
                        bb = o2 + nn
                    bounds.append(bb)
                bounds.append(na)
                half_slots = max(-(-(bounds[pi + 1] - bounds[pi]) // 128)
                                 for pi in range(NPIECE))
                for piece in range(NPIECE):
                    p_lo, p_hi = bounds[piece], bounds[piece + 1]
                    if p_lo >= p_hi:
                        continue
                    a_sb = bigp.tile([128, half_slots, DIM], f32,
                                     name=f"a{k}_{piece}", tag="abuf")
                    a16 = bigp.tile([128, half_slots, 2 * DIM], f16,
                                    name=f"a16_{k}_{piece}", tag="abuf16")
                    if na < 16384:  # small-scale sim: avoid uninit-pad reads
                        nc.vector.tensor_copy(out=a16[:, :, DIM:2 * DIM],
                                              in_=a16[:, :, 0:DIM]) if False else \
                            nc.gpsimd.memset(a16[:], 0)
                    for (w, o2, nn) in acalls:
                        if o2 >= p_hi or o2 + nn <= p_lo:
                            continue
                        assert o2 >= p_lo and o2 + nn <= p_hi, "call straddles piece"
                        ol = o2 - p_lo
                        nc.gpsimd.dma_gather(
                            out_ap=a_sb[:, ol // 128:(ol + nn) // 128, :],
                            in_ap=ent[WIN * w: min(WIN * (w + 1), N_ENTITY), :],
                            idxs_ap=ia[:, o2 // 16:(o2 + nn) // 16],
                            num_idxs=nn, num_idxs_reg=nn, elem_size=DIM,
                            queue_num=_nextq(), single_packet=False)
                    nc.vector.tensor_copy(
                        out=a16[:, 0:(p_hi - p_lo) // 128, 0:DIM],
                        in_=a_sb[:, 0:(p_hi - p_lo) // 128, :])
                    nc.sync.dma_start(
                        out=stage[k][p_lo:p_hi, :].rearrange("(s p) d -> p s d", p=128),
                        in_=a16[:, 0:(p_hi - p_lo) // 128, :])

                # ---- index/relation loads ----
                ih = ixp.tile([128, n_col // 16], i16, name=f"ih{k}", tag="ih")
                nc.sync.dma_start(out=ih[:], in_=idxh[k][:])
                it = ixp.tile([128, n_col // 16], i16, name=f"it{k}", tag="it")
                nc.sync.dma_start(out=it[:], in_=idxt[k][:])

                gs = kp.tile([128, T, NB, 65], f16, name=f"gs{k}", tag="gs")
                exh = kp.tile([128, NB, T], f32, name=f"exh{k}", tag="exh")
                nmh = kp.tile([128, NB, NH], f32, name=f"nmh{k}", tag="nmh")
                esh = kp.tile([128, NB, NH], f32, name=f"esh{k}", tag="esh")
                psum_t = kp.tile([128, NB, NH, DIM], f32, name=f"pst{k}", tag="pst")
                if tw == "u" and l == 0:
                    uacc = kp.tile([64, b_core], f32)

                for hf in range(NH):
                    cbase = hf * COLS_H
                    h_bm = bgp.tile([128, COLS_H // 128, 2 * DIM], f16,
                                    name=f"hb{k}_{hf}", tag="hbm")
                    te_bm = bgp.tile([128, COLS_H // 128, 2 * DIM], f16,
                                     name=f"tb{k}_{hf}", tag="tbm")
                    for q in range(2):
                        o2 = cbase + q * GQ
                        nc.gpsimd.dma_gather(
                            out_ap=h_bm[:, q * GQ // 128:(q + 1) * GQ // 128, :],
                            in_ap=stage[k][:],
                            idxs_ap=ih[:, o2 // 16:(o2 + GQ) // 16],
                            num_idxs=GQ, num_idxs_reg=GQ, elem_size=2 * DIM,
                            queue_num=_nextq(), single_packet=False)
                        nc.gpsimd.dma_gather(
                            out_ap=te_bm[:, q * GQ // 128:(q + 1) * GQ // 128, :],
                            in_ap=stage[k][:],
                            idxs_ap=it[:, o2 // 16:(o2 + GQ) // 16],
                            num_idxs=GQ, num_idxs_reg=GQ, elem_size=2 * DIM,
                            queue_num=_nextq(), single_packet=False)

                    for tt in range(TH):
                        t = hf * TH + tt
                        tcol = t * (128 * NB)
                        xps = psA.tile([64, 128 * NB], f16, space="PSUM",
                                       name="xps", tag="xps")
                        for q in range(NB):
                            nc.tensor.transpose(
                                out=xps[:, q * 128:(q + 1) * 128],
                                in_=h_bm[:, tt * NB + q, 0:DIM], identity=ident16[:])
                        x_fm = wp.tile([64, 128 * NB], f16, name="xfm", tag="xfm")
                        nc.scalar.activation(out=x_fm[:], in_=xps[:], func=AF.Copy)
                        rrow = wp.tile([1, 128 * NB], f16, name="rr", tag="rr")
                        nc.sync.dma_start(out=rrow[:],
                                          in_=rflat[k][:, tcol:tcol + 128 * NB])
                        rps = psA.tile([N_RELATION, 128 * NB], f32, space="PSUM",
                                       name="rps", tag="rps")
                        nc.tensor.matmul(out=rps[:], lhsT=ones1[:],
                                         rhs=rrow[:], start=True, stop=True)
                        oh = wp.tile([N_RELATION, 128 * NB], f16, name="oh", tag="oh")
                        nc.vector.tensor_scalar(
                            out=oh[:], in0=rps[:], scalar1=iota_sb[:, 0:1],
                            scalar2=None, op0=ALU.is_equal)
                        p1 = psB.tile([128, 128 * NB], f32, space="PSUM",
                                      name="p1", tag="p1")
                        nc.tensor.matmul(out=p1[:], lhsT=w1h_sb[:], rhs=x_fm[:],
                                         start=True, stop=False)
                        nc.tensor.matmul(out=p1[:], lhsT=r1p_sb[:], rhs=oh[:],
                                         start=False, stop=True)
                        lflip = wp.tile([128, 128 * NB], f16, name="lflip", tag="lflip")
                        nc.vector.tensor_scalar(
                            out=lflip[0:64, :], in0=p1[0:64, :], scalar1=0.0,
                            scalar2=None, op0=ALU.max)
                        r1s = wp.tile([64, 128 * NB], f16, name="r1s", tag="r1s")
                        nc.scalar.activation(out=r1s[:], in_=p1[64:128, :], func=AF.Relu)
                        p2 = psA.tile([64, 128 * NB], f32, space="PSUM",
                                      name="p2", tag="p2")
                        nc.tensor.matmul(out=p2[:], lhsT=w2_sb[:], rhs=r1s[:],
                                         start=True, stop=True)
                        nc.vector.tensor_scalar(
                            out=lflip[64:128, :], in0=p2[:], scalar1=0.0,
                            scalar2=None, op0=ALU.max)
                        pf = psF.tile([128, NB, 65], f32, space="PSUM",
                                      name="pf", tag="pf")
                        for q in range(NB):
                            nc.tensor.matmul(out=pf[:, q, :],
                                             lhsT=lflip[:, q * 128:(q + 1) * 128],
                                             rhs=wflip_sb[:], start=True, stop=True)
                        nc.scalar.activation(out=gs[:, t, :, :], in_=pf[:],
                                             func=AF.Sigmoid)
                        if tw == "u" and l == 0:
                            if t == 0:
                                nc.vector.tensor_copy(out=uacc[:], in_=x_fm[:])
                            else:
                                nc.vector.tensor_add(out=uacc[:], in0=uacc[:],
                                                     in1=x_fm[:])

                    # ---- per-half unnormalized weighted sum ----
                    for q in range(NB):
                        s3v = gs[:, hf * TH:(hf + 1) * TH, q, 64:65]
                        nm = wp.tile([128, 1], f32, name="nm", tag="nm")
                        nc.vector.tensor_reduce(out=nm[:], in_=s3v, axis=AX.XY,
                                                op=ALU.max, negate=True)
                        ex = wp.tile([128, TH], f32, name="ex", tag="ex")
                        es = wp.tile([128, 1], f32, name="es", tag="es")
                        nc.scalar.activation(out=ex[:], in_=s3v, func=AF.Exp,
                                             bias=nm[:, 0:1], accum_out=es[:])
                        nc.vector.tensor_copy(out=exh[:, q, hf * TH:(hf + 1) * TH],
                                              in_=ex[:])
                        nc.vector.tensor_copy(out=nmh[:, q, hf:hf + 1], in_=nm[:])
                        nc.vector.tensor_copy(out=esh[:, q, hf:hf + 1], in_=es[:])
                        wmul = wp.tile([128, TH, DIM], f32, name="wmul", tag="wmul")
                        nc.vector.tensor_tensor(
                            out=wmul[:],
                            in0=te_bm[:].rearrange("p (t q) d -> p t q d", q=NB)[:, :, q, 0:DIM],
                            in1=ex[:, :, None].to_broadcast([128, TH, DIM]),
                            op=ALU.mult)
                        pmul = wp.tile([128, TH, DIM], f32, name="pmul", tag="pmul")
                        nc.vector.tensor_tensor(
                            out=pmul[:], in0=wmul[:],
                            in1=gs[:, hf * TH:(hf + 1) * TH, q, 0:64],
                            op=ALU.mult)
                        f8 = wp.tile([128, TH // 2, DIM], f32, name="f8", tag="f8")
                        nc.vector.tensor_add(out=f8[:], in0=pmul[:, 0:TH // 2, :],
                                             in1=pmul[:, TH // 2:TH, :])
                        f4 = wp.tile([128, TH // 4, DIM], f32, name="f4", tag="f4")
                        nc.vector.tensor_add(out=f4[:], in0=f8[:, 0:TH // 4, :],
                                             in1=f8[:, TH // 4:TH // 2, :])
                        f2 = wp.tile([128, 2, DIM], f32, name="f2", tag="f2")
                        nc.vector.tensor_add(out=f2[:], in0=f4[:, 0:2, :],
                                             in1=f4[:, 2:4, :])
                        nc.vector.tensor_add(out=psum_t[:, q, hf, :],
                                             in0=f2[:, 0, :], in1=f2[:, 1, :])

                # ---- combine halves with softmax normalization ----
                for q in range(NB):
                    nmall = wp.tile([128, 1], f32, name="nmall", tag="nmall")
                    nc.vector.tensor_reduce(out=nmall[:], in_=nmh[:, q, :],
                                            axis=AX.X, op=ALU.min)
                    dif = wp.tile([128, NH], f32, name="dif", tag="dif")
                    nc.vector.tensor_scalar(
                        out=dif[:], in0=nmh[:, q, :], scalar1=-1.0,
                        scalar2=nmall[:, 0:1], op0=ALU.mult, op1=ALU.add)
                    sc = wp.tile([128, NH], f32, name="sc", tag="sc")
                    nc.scalar.activation(out=sc[:], in_=dif[:], func=AF.Exp)
                    stmp = wp.tile([128, NH], f32, name="stmp", tag="stmp")
                    nc.vector.tensor_tensor(out=stmp[:], in0=esh[:, q, :], in1=sc[:],
                                            op=ALU.mult)
                    tot = wp.tile([128, 1], f32, name="tot", tag="tot")
                    nc.vector.tensor_reduce(out=tot[:], in_=stmp[:], axis=AX.X,
                                            op=ALU.add)
                    rec = wp.tile([128, 1], f32, name="rec", tag="rec")
                    nc.vector.reciprocal(out=rec[:], in_=tot[:])
                    pw = wp.tile([128, NH, DIM], f32, name="pw", tag="pw")
                    nc.vector.tensor_tensor(
                        out=pw[:], in0=psum_t[:, q, :, :],
                        in1=sc[:, :, None].to_broadcast([128, NH, DIM]), op=ALU.mult)
                    osum = wp.tile([128, DIM], f32, name="osum", tag="osum")
                    nc.vector.tensor_add(out=osum[:], in0=pw[:, 0, :], in1=pw[:, 1, :])
                    o = kp.tile([128, DIM], f32, name=f"otl{k}_{q}", tag=f"otl{k}_{q}")
                    nc.vector.tensor_scalar(
                        out=o[:], in0=osum[:], scalar1=rec[:, 0:1], scalar2=2.0,
                        op0=ALU.mult, op1=ALU.mult)
                    otl[(k, q)] = o

                if tw == "u" and l == 0:
                    for q in range(NB):
                        ups = psA.tile([128, 64], f32, space="PSUM",
                                       name="ups", tag="ups")
                        nc.tensor.transpose(out=ups[:],
                                            in_=uacc[:, q * 128:(q + 1) * 128],
                                            identity=ident[0:64, 0:64])
                        ub = kp.tile([128, 64], f32, name=f"ub{q}", tag=f"ub{q}")
                        nc.scalar.activation(out=ub[:], in_=ups[:], func=AF.Copy)
                        uorig_bm[q] = ub

            # ---- scores ----
            sc_all = kp.tile([128, NB], f32)
            for q in range(NB):
                m = wp.tile([128, DIM], f32, name="m", tag="m")
                nc.vector.tensor_tensor(out=m[:], in0=uorig_bm[q][:],
                                        in1=iorig[:, q, :], op=ALU.mult)
                acc = wp.tile([128, DIM], f32, name="macc", tag="macc")
                nc.vector.tensor_scalar(out=acc[:], in0=m[:], scalar1=1.0 / T,
                                        scalar2=None, op0=ALU.mult)
                for ku, ki in ((0, 2), (1, 3)):
                    mu = wp.tile([128, DIM], f32, name="mu", tag="mu")
                    nc.vector.tensor_tensor(out=mu[:], in0=otl[(ku, q)][:],
                                            in1=otl[(ki, q)][:], op=ALU.mult)
                    nc.vector.tensor_add(out=acc[:], in0=acc[:], in1=mu[:])
                ssum = wp.tile([128, 1], f32, name="ssum", tag="ssum")
                nc.vector.tensor_reduce(out=ssum[:], in_=acc[:], axis=AX.X,
                                        op=ALU.add)
                nc.scalar.activation(out=sc_all[:, q:q + 1], in_=ssum[:],
                                     func=AF.Sigmoid)
            nc.sync.dma_start(out=scores_hbm.rearrange("(s p) -> p s", p=128),
                              in_=sc_all[:])
    nc.compile()
    return nc


def _make_in_maps(inputs):
    b_core, n_col, nwin, caps, na = _dims()
    ent = np.asarray(inputs["ent_emb"], np.float32)
    rel = np.asarray(inputs["rel_emb"], np.float32)
    att_w1 = np.asarray(inputs["att_w1"], np.float32)
    att_w2 = np.asarray(inputs["att_w2"], np.float32)
    att_w3 = np.asarray(inputs["att_w3"], np.float32)
    gate_w1 = np.asarray(inputs["gate_w1"], np.float32)
    gate_w2 = np.asarray(inputs["gate_w2"], np.float32)
    items = np.asarray(inputs["items"]).astype(np.int64)
    idx6 = {n: np.asarray(inputs[n]).astype(np.int64)
            for n in ("user_h", "user_r", "user_t", "item_h", "item_r", "item_t")}

    w1h = np.concatenate([gate_w1[:DIM], att_w1[:DIM]], axis=1).astype(np.float16)
    r1p = (rel @ np.concatenate([gate_w1[DIM:], att_w1[DIM:]], axis=1)).astype(np.float16)
    wflip = np.zeros((128, 65), np.float16)
    wflip[0:64, 0:64] = gate_w2.astype(np.float16)
    wflip[64:128, 64:65] = att_w3.astype(np.float16)
    iota = np.arange(N_RELATION, dtype=np.float32).reshape(N_RELATION, 1)

    in_maps = []
    for c in range(N_CORES):
        sl = slice(c * b_core, (c + 1) * b_core)
        im = {
            "ent": ent, "w1h": w1h, "r1p": r1p, "w2": att_w2.astype(np.float16),
            "wflip": wflip, "iota": iota,
            "items16": _wrap_idx16(items[sl].astype(np.int16)),
        }
        for k, (tw, l) in enumerate(TL_LIST):
            pre = "user" if tw == "u" else "item"
            h = idx6[f"{pre}_h"][l, sl].T.ravel()
            t = idx6[f"{pre}_t"][l, sl].T.ravel()
            r = idx6[f"{pre}_r"][l, sl].T.ravel()
            ia, hl, tl_ = _host_prep_tl(h, t)
            im[f"idxA{k}"] = _wrap_idx16(ia)
            im[f"idxh{k}"] = _wrap_idx16(hl)
            im[f"idxt{k}"] = _wrap_idx16(tl_)
            im[f"rflat{k}"] = r.astype(np.float16).reshape(1, n_col)
        in_maps.append(im)
    return in_maps


def kernel(**inputs):
    global _NC_CACHE
    import os
    from concourse.bass_utils import run_bass_kernel_spmd

    if _NC_CACHE is None:
        _NC_CACHE = _build_program()
    nc = _NC_CACHE
    in_maps = _make_in_maps(inputs)
    trace = bool(int(os.environ.get("CKAN_TRACE", "0")))
    res = run_bass_kernel_spmd(nc, in_maps, core_ids=list(range(N_CORES)),
                               trace=trace)
    if trace and res.exec_time_ns is not None:
        print(f"HW exec time: {res.exec_time_ns} ns")
    if trace and res.instructions_and_trace is not None:
        print(f"trace path: {res.instructions_and_trace[1]}")
    b_core = B // N_CORES
    out = np.concatenate([res.results[c]["scores"] for c in range(N_CORES)])
    return out.astype(np.float32)



# revision 5
# speedup vs baseline: 1.1321x; 1.1321x over previous
"""CKAN (gnn_message_passing) Trainium2 kernel, v2.

Data-parallel over 8 NeuronCores (512 batch rows each), no collectives.
Entity rows are fetched in two levels to fit dma_gather's int16 index
limit, but unlike v1 the compact unique table lives in SBUF:
  phase A: sorted-unique h∪t rows gathered from a host-padded f16 table
           (row = [e(64 f16) | pad]) with windowed calls directly into a
           [128, na/128, 128] SBUF tile (row i -> partition i%128,
           slot i//128) -- no HBM staging, no dtype-convert copies;
  phase B: per-(b,t) rows fetched with SBUF-source transpose-gathers,
           which deliver rows FEATURE-major: h rows land as the MLP's
           rhs directly (no PE transposes), t rows are transposed back
           to batch-major on the PE per 128-column block.
The relation contribution enters the layer-1 matmul as a host-shipped
one-hot [32, n_col] that is DMA'd into partitions 64:96 of the same
gather-output tile, so layer 1 is a single [96->128] matmul against
w1r = [[gate_w1h | att_w1h], rel_emb @ [gate_w1d | att_w1d]].
gate2 and att3 run as "flip" matmuls producing batch-major [g2 | s3].
Softmax over the 32 neighbors is computed in 4 quarter-partials (8
neighbors each) that are renormalized and combined at the end; the
weighted sums use one stride-permuted tensor_reduce per (quarter,
b-block) instead of log-tree adds.
"""
import sys
sys.path.insert(0, '/opt/trn_rl_repo')
import numpy as np

# ---- problem dims (overridable for small-scale sim tests) ----
DIM = 64
N_ENTITY = 100000
N_RELATION = 32
N_LAYER = 2
B = 4096
T = 32
N_CORES = 8
WIN = 32768
NW_CAP = None   # computed in _dims()
_NC_CACHE = None


def _dims():
    b_core = B // N_CORES
    n_col = b_core * T
    nwin = (N_ENTITY + WIN - 1) // WIN
    if NW_CAP is not None:
        caps = list(NW_CAP)
    else:
        # expected uniques per full window for 2*n_col draws over N_ENTITY,
        # with ~8 sigma headroom, rounded to 128
        import math
        caps = []
        for w in range(nwin):
            width = min(WIN, N_ENTITY - WIN * w)
            mean = width * (1.0 - (1.0 - 1.0 / N_ENTITY) ** (2 * n_col))
            cap = int(mean + 8 * math.sqrt(max(mean, 1.0)) + 64)
            caps.append(-(-cap // 128) * 128)
    na = sum(caps)
    assert na % 128 == 0 and na <= 32767
    return b_core, n_col, nwin, caps, na


def _wrap_idx16(a):
    """int16 vector -> dma_gather idx layout [128, ceil(n/16)]."""
    a = np.asarray(a, dtype=np.int16)
    n = len(a)
    pad = (-n) % 16
    if pad:
        a = np.concatenate([a, np.full(pad, -1, np.int16)])
    w = a.reshape(-1, 16).T.copy()
    return np.tile(w, (8, 1))


def _host_prep_tl(h_flat, t_flat):
    b_core, n_col, nwin, caps, na = _dims()
    uni = np.unique(np.concatenate([h_flat, t_flat]))
    val_to_pos = np.full(N_ENTITY, -1, np.int32)
    idxA_parts = []
    off = 0
    for w in range(nwin):
        lo, hi = WIN * w, min(WIN * (w + 1), N_ENTITY)
        seg = uni[(uni >= lo) & (uni < hi)]
        cap = caps[w]
        assert len(seg) <= cap, f"window {w} overflow: {len(seg)} > {cap}"
        val_to_pos[seg] = off + np.arange(len(seg), dtype=np.int32)
        assert len(seg) == 0 or off + len(seg) - 1 <= 32767, "position overflow"
        fill = (seg[-1] - lo) if len(seg) else 0
        seg_l = np.concatenate([(seg - lo).astype(np.int16),
                                np.full(cap - len(seg), fill, np.int16)])
        idxA_parts.append(seg_l)
        off += cap
    idxA = np.concatenate(idxA_parts)
    h_loc = val_to_pos[h_flat]
    t_loc = val_to_pos[t_flat]
    assert (h_loc >= 0).all() and (t_loc >= 0).all()
    return idxA, h_loc.astype(np.int16), t_loc.astype(np.int16)


TL_LIST = [("u", 0), ("u", 1), ("i", 0), ("i", 1)]
NQ = 4            # softmax quarter-partials


def _build_program(debug=True):
    import concourse.bacc as bacc
    import concourse.tile as tile
    from concourse import mybir
    from concourse.masks import make_identity
    from concourse import tile_sem_assignment as tsa

    # Tile assigns DMASW sem lanes round-robin in scheduled order, but each
    # lane is hardware-locked to SWDGE queue (lane % 4).  Force gather
    # instructions onto lanes consistent with their queue_num.
    if not getattr(tsa.TileClockTick, "_gather_lane_patched", False):
        _orig_assign_tick = tsa.TileClockTick._assign_tick

        def _patched_assign_tick(self, inst):
            if isinstance(inst, mybir.InstDMAGatherAnt):
                q = inst.queue_num
                tog = getattr(self, "_gather_lane_toggle", None)
                if tog is None:
                    tog = self._gather_lane_toggle = {}
                k = tog.get(q, 0)
                tog[q] = k ^ 1
                saved = self.next_sw_dma_idx
                self.next_sw_dma_idx = q + 4 * k
                try:
                    return _orig_assign_tick(self, inst)
                finally:
                    self.next_sw_dma_idx = saved
            return _orig_assign_tick(self, inst)

        tsa.TileClockTick._assign_tick = _patched_assign_tick
        tsa.TileClockTick._gather_lane_patched = True

    f32 = mybir.dt.float32
    f16 = mybir.dt.float16
    i16 = mybir.dt.int16
    AF = mybir.ActivationFunctionType
    ALU = mybir.AluOpType
    AX = mybir.AxisListType

    b_core, n_col, nwin, caps, na = _dims()
    NB = b_core // 128               # b-chunks per column tile
    TQ = T // NQ                     # t per quarter (8)
    TILE = 128 * NB                  # columns per tile (512)
    COLS_Q = TILE * TQ               # columns per quarter (4096)
    na_slots = na // 128
    GMAX = 4096                      # max idx per gather call

    nc = bacc.Bacc("TRN2", target_bir_lowering=False, debug=debug,
                   num_swdge_queues=4)
    _qctr = [0]

    def _nextq():
        q = _qctr[0] % 4
        _qctr[0] += 1
        return q

    entp = nc.dram_tensor("entp", [N_ENTITY, 2 * DIM], f16, kind="ExternalInput")
    w1r = nc.dram_tensor("w1r", [DIM + N_RELATION, 128], f16, kind="ExternalInput")
    w2 = nc.dram_tensor("w2", [64, 64], f16, kind="ExternalInput")
    wflip = nc.dram_tensor("wflip", [128, 65], f16, kind="ExternalInput")
    items16 = nc.dram_tensor("items16", [128, max(b_core // 16, 1)], i16,
                             kind="ExternalInput")
    idxA, idxQ, ohr = {}, {}, {}
    for k in range(4):
        idxA[k] = nc.dram_tensor(f"idxA{k}", [128, na // 16], i16,
                                 kind="ExternalInput")
        idxQ[k] = nc.dram_tensor(f"idxQ{k}", [128, 2 * n_col // 16], i16,
                                 kind="ExternalInput")
        ohr[k] = nc.dram_tensor(f"ohr{k}", [N_RELATION, n_col], f16,
                                kind="ExternalInput")
    scores_hbm = nc.dram_tensor("scores", [b_core], f32, kind="ExternalOutput")

    with tile.TileContext(nc) as tc:
        with (
            tc.tile_pool(name="const", bufs=1) as cp,
            tc.tile_pool(name="tab", bufs=1) as tabp,
            tc.tile_pool(name="xt", bufs=2) as xtp,
            tc.tile_pool(name="te", bufs=2) as tep,
            tc.tile_pool(name="gsq", bufs=2) as gsp,
            tc.tile_pool(name="work", bufs=2) as wp,
            tc.tile_pool(name="idx", bufs=2) as ixp,
            tc.tile_pool(name="keep", bufs=1) as kp,
            tc.tile_pool(name="ps1", bufs=2, space="PSUM") as ps1,
            tc.tile_pool(name="ps2", bufs=2, space="PSUM") as ps2,
            tc.tile_pool(name="psF", bufs=2, space="PSUM") as psF,
            tc.tile_pool(name="psT", bufs=2, space="PSUM") as psT,
        ):
            ident = cp.tile([128, 128], f32)
            make_identity(nc, ident[:])
            ident16 = cp.tile([128, 128], f16)
            nc.vector.tensor_copy(out=ident16[:], in_=ident[:])
            w1r_sb = cp.tile([DIM + N_RELATION, 128], f16)
            nc.sync.dma_start(out=w1r_sb[:], in_=w1r[:])
            w2_sb = cp.tile([64, 64], f16)
            nc.sync.dma_start(out=w2_sb[:], in_=w2[:])
            wflip_sb = cp.tile([128, 65], f16)
            nc.sync.dma_start(out=wflip_sb[:], in_=wflip[:])

            items_sb = cp.tile([128, max(b_core // 16, 1)], i16)
            nc.sync.dma_start(out=items_sb[:], in_=items16[:])
            iorig = kp.tile([128, NB, 2 * DIM], f16)
            nc.gpsimd.dma_gather(
                out_ap=iorig[:], in_ap=entp[:], idxs_ap=items_sb[:],
                num_idxs=b_core, num_idxs_reg=b_core, elem_size=2 * DIM,
                queue_num=_nextq(), single_packet=False)

            otl = {}            # k -> [128, NB, DIM] f32 layer outputs
            uorig_bm = None

            # flat list of (window, offset, count) phase-A calls, <=GMAX idx
            acalls = []
            off = 0
            for w in range(nwin):
                nw_ = caps[w]
                nsplit = max(1, -(-nw_ // GMAX))
                step = -(-(-(-nw_ // nsplit)) // 128) * 128
                done = 0
                while done < nw_:
                    nn = min(step, nw_ - done)
                    acalls.append((w, off + done, nn))
                    done += nn
                off += nw_

            for k, (tw, l) in enumerate(TL_LIST):
                # ---- phase A: unique rows -> SBUF table ----
                ia = ixp.tile([128, na // 16], i16, name=f"ia{k}", tag="ia")
                nc.sync.dma_start(out=ia[:], in_=idxA[k][:])
                table = tabp.tile([128, na_slots, 2 * DIM], f16,
                                  name=f"table{k}", tag="table")
                for (w, o2, nn) in acalls:
                    nc.gpsimd.dma_gather(
                        out_ap=table[:, o2 // 128:(o2 + nn) // 128, :],
                        in_ap=entp[WIN * w: min(WIN * (w + 1), N_ENTITY), :],
                        idxs_ap=ia[:, o2 // 16:(o2 + nn) // 16],
                        num_idxs=nn, num_idxs_reg=nn, elem_size=2 * DIM,
                        queue_num=_nextq(), single_packet=False)

                iq = ixp.tile([128, 2 * n_col // 16], i16, name=f"iq{k}",
                              tag="iq")
                nc.sync.dma_start(out=iq[:], in_=idxQ[k][:])

                gs_q = {}
                exh = kp.tile([128, NB, T], f16, name=f"exh{k}", tag="exh")
                nmh = kp.tile([128, NB, NQ], f32, name=f"nmh{k}", tag="nmh")
                esh = kp.tile([128, NB, NQ], f32, name=f"esh{k}", tag="esh")
                psum_t = kp.tile([128, NB, NQ, DIM], f32, name=f"pst{k}",
                                 tag="pst")
                if tw == "u" and l == 0:
                    uacc = kp.tile([64, b_core], f32)

                for qu in range(NQ):
                    cbase = qu * COLS_Q
                    xt = xtp.tile([128, 1, 2 * COLS_Q], f16,
                                  name=f"xt{k}_{qu}", tag="xt")
                    # h rows -> feature-major cols [0, COLS_Q)
                    # t rows -> feature-major cols [COLS_Q, 2*COLS_Q)
                    iqo = qu * (2 * COLS_Q) // 16
                    for part in range(2):
                        nc.gpsimd.dma_gather(
                            out_ap=xt[:, :, part * COLS_Q:(part + 1) * COLS_Q],
                            in_ap=table[:],
                            idxs_ap=iq[:, iqo + part * COLS_Q // 16:
                                       iqo + (part + 1) * COLS_Q // 16],
                            num_idxs=COLS_Q, num_idxs_reg=COLS_Q,
                            elem_size=2 * DIM, transpose=True,
                            sbuf_tokens_per_rank=128,
                            sbuf_free_dim_per_rank=4 * DIM,
                            queue_num=_nextq(), single_packet=False)
                    # one-hot relations into partitions 64:96 of the h cols
                    nc.sync.dma_start(
                        out=xt[DIM:DIM + N_RELATION, 0, 0:COLS_Q],
                        in_=ohr[k][:, cbase:cbase + COLS_Q])

                    te_sb = tep.tile([128, TQ, NB, DIM], f16,
                                     name=f"te{k}_{qu}", tag="te")
                    gsq = gsp.tile([128, TQ, NB, 65], f16,
                                   name=f"gs{k}_{qu}", tag="gsq")
                    gs_q[qu] = gsq
                    for tt in range(TQ):
                        cols = slice(tt * TILE, (tt + 1) * TILE)
                        p1 = ps1.tile([128, TILE], f32, space="PSUM",
                                      name="p1", tag="p1")
                        nc.tensor.matmul(out=p1[:],
                                         lhsT=w1r_sb[:],
                                         rhs=xt[0:DIM + N_RELATION, 0, cols],
                                         start=True, stop=True)
                        # t rows back to batch-major via PE transpose
                        pt = psT.tile([128, NB, DIM], f16, space="PSUM",
                                      name="pt", tag="pt")
                        for q in range(NB):
                            c0 = COLS_Q + tt * TILE + q * 128
                            nc.tensor.transpose(
                                out=pt[:, q, :],
                                in_=xt[0:DIM, 0, c0:c0 + 128],
                                identity=ident16[0:DIM, 0:DIM])
                        nc.scalar.activation(out=te_sb[:, tt, :, :],
                                             in_=pt[:], func=AF.Copy)
                        lflip = wp.tile([128, TILE], f16, name="lflip",
                                        tag="lflip")
                        nc.vector.tensor_scalar(
                            out=lflip[0:64, :], in0=p1[0:64, :], scalar1=0.0,
                            scalar2=None, op0=ALU.max)
                        r1s = wp.tile([64, TILE], f16, name="r1s", tag="r1s")
                        nc.scalar.activation(out=r1s[:], in_=p1[64:128, :],
                                             func=AF.Relu)
                        p2 = ps2.tile([64, TILE], f32, space="PSUM",
                                      name="p2", tag="p2")
                        nc.tensor.matmul(out=p2[:], lhsT=w2_sb[:], rhs=r1s[:],
                                         start=True, stop=True)
                        nc.vector.tensor_scalar(
                            out=lflip[64:128, :], in0=p2[:], scalar1=0.0,
                            scalar2=None, op0=ALU.max)
                        pf = psF.tile([128, NB, 65], f32, space="PSUM",
                                      name="pf", tag="pf")
                        for q in range(NB):
                            nc.tensor.matmul(out=pf[:, q, :],
                                             lhsT=lflip[:, q * 128:(q + 1) * 128],
                                             rhs=wflip_sb[:],
                                             start=True, stop=True)
                        nc.scalar.activation(out=gsq[:, tt, :, :], in_=pf[:],
                                             func=AF.Sigmoid)

                    # ---- per-quarter softmax partials + weighted sums ----
                    nc.vector.tensor_reduce(
                        out=nmh[:, :, qu:qu + 1],
                        in_=gsq[:, :, :, 64].rearrange("p t q -> p q t"),
                        axis=AX.X, op=ALU.max, negate=True)
                    for q in range(NB):
                        nc.scalar.activation(
                            out=exh[:, q, qu * TQ:(qu + 1) * TQ],
                            in_=gsq[:, :, q, 64:65], func=AF.Exp,
                            bias=nmh[:, q, qu:qu + 1],
                            accum_out=esh[:, q, qu:qu + 1])
                        wmul = wp.tile([128, TQ, DIM], f16, name="wmul",
                                       tag="wmul")
                        nc.vector.tensor_tensor(
                            out=wmul[:], in0=te_sb[:, :, q, :],
                            in1=exh[:, q, qu * TQ:(qu + 1) * TQ, None]
                                .to_broadcast([128, TQ, DIM]),
                            op=ALU.mult)
                        pmul = wp.tile([128, TQ, DIM], f16, name="pmul",
                                       tag="pmul")
                        nc.vector.tensor_tensor(
                            out=pmul[:], in0=wmul[:],
                            in1=gsq[:, :, q, 0:64], op=ALU.mult)
                        nc.vector.tensor_reduce(
                            out=psum_t[:, q, qu, :],
                            in_=pmul[:].rearrange("p t d -> p d t"),
                            axis=AX.X, op=ALU.add)

                    if tw == "u" and l == 0:
                        ured = wp.tile([64, b_core], f32, name="ured",
                                       tag="ured")
                        nc.vector.tensor_reduce(
                            out=ured[:],
                            in_=xt[0:DIM, 0, 0:COLS_Q]
                                .rearrange("p (t b) -> p b t", b=TILE),
                            axis=AX.X, op=ALU.add)
                        if qu == 0:
                            nc.vector.tensor_copy(out=uacc[:], in_=ured[:])
                        else:
                            nc.vector.tensor_add(out=uacc[:], in0=uacc[:],
                                                 in1=ured[:])

                # ---- combine quarters with softmax renormalization ----
                nmall = wp.tile([128, NB, 1], f32, name="nmall", tag="nmall")
                nc.vector.tensor_reduce(out=nmall[:], in_=nmh[:],
                                        axis=AX.X, op=ALU.min)
                dif = wp.tile([128, NB, NQ], f32, name="dif", tag="dif")
                nc.vector.tensor_tensor(
                    out=dif[:], in0=nmall[:].to_broadcast([128, NB, NQ]),
                    in1=nmh[:], op=ALU.subtract)
                sc = wp.tile([128, NB, NQ], f32, name="sc", tag="sc")
                nc.scalar.activation(out=sc[:], in_=dif[:], func=AF.Exp)
                stmp = wp.tile([128, NB, NQ], f32, name="stmp", tag="stmp")
                nc.vector.tensor_tensor(out=stmp[:], in0=esh[:], in1=sc[:],
                                        op=ALU.mult)
                tot = wp.tile([128, NB, 1], f32, name="tot", tag="tot")
                nc.vector.tensor_reduce(out=tot[:], in_=stmp[:], axis=AX.X,
                                        op=ALU.add)
                rec = wp.tile([128, NB, 1], f32, name="rec", tag="rec")
                nc.vector.reciprocal(out=rec[:], in_=tot[:])
                pw = wp.tile([128, NB, NQ, DIM], f32, name="pw", tag="pw")
                nc.vector.tensor_tensor(
                    out=pw[:], in0=psum_t[:],
                    in1=sc[:, :, :, None]
                        .to_broadcast([128, NB, NQ, DIM]),
                    op=ALU.mult)
                osum = wp.tile([128, NB, DIM], f32, name="osum", tag="osum")
                nc.vector.tensor_reduce(
                    out=osum[:], in_=pw[:].rearrange("p q h d -> p q d h"),
                    axis=AX.X, op=ALU.add)
                o = kp.tile([128, NB, DIM], f32, name=f"otl{k}",
                            tag=f"otl{k}")
                nc.vector.tensor_tensor(
                    out=o[:], in0=osum[:],
                    in1=rec[:].to_broadcast([128, NB, DIM]), op=ALU.mult)
                otl[k] = o

                if tw == "u" and l == 0:
                    put = psT.tile([128, NB, DIM], f32, space="PSUM",
                                   name="put", tag="pt")
                    for q in range(NB):
                        nc.tensor.transpose(
                            out=put[:, q, :],
                            in_=uacc[:, q * 128:(q + 1) * 128],
                            identity=ident[0:64, 0:64])
                    ub = kp.tile([128, NB, DIM], f32, name="ub", tag="ub")
                    nc.scalar.activation(out=ub[:], in_=put[:], func=AF.Copy)
                    uorig_bm = ub

            # ---- scores (2.0 gate scale twice, 1/T origin mean) ----
            m = wp.tile([128, NB, DIM], f32, name="m", tag="m")
            nc.vector.tensor_tensor(out=m[:], in0=uorig_bm[:],
                                    in1=iorig[:, :, 0:DIM], op=ALU.mult)
            acc = wp.tile([128, NB, DIM], f32, name="macc", tag="macc")
            nc.vector.tensor_scalar(out=acc[:], in0=m[:], scalar1=1.0 / T,
                                    scalar2=None, op0=ALU.mult)
            for ku, ki in ((0, 2), (1, 3)):
                mu = wp.tile([128, NB, DIM], f32, name="mu", tag="mu")
                nc.vector.tensor_tensor(out=mu[:], in0=otl[ku][:],
                                        in1=otl[ki][:], op=ALU.mult)
                nc.vector.tensor_scalar(out=mu[:], in0=mu[:], scalar1=4.0,
                                        scalar2=None, op0=ALU.mult)
                nc.vector.tensor_add(out=acc[:], in0=acc[:], in1=mu[:])
            ssum = wp.tile([128, NB, 1], f32, name="ssum", tag="ssum")
            nc.vector.tensor_reduce(out=ssum[:], in_=acc[:], axis=AX.X,
                                    op=ALU.add)
            sc_all = kp.tile([128, NB], f32)
            nc.scalar.activation(out=sc_all[:], in_=ssum[:, :, 0],
                                 func=AF.Sigmoid)
            nc.sync.dma_start(out=scores_hbm.rearrange("(s p) -> p s", p=128),
                              in_=sc_all[:])
    nc.compile()
    return nc


def _make_in_maps(inputs):
    b_core, n_col, nwin, caps, na = _dims()
    ent = np.asarray(inputs["ent_emb"], np.float32)
    rel = np.asarray(inputs["rel_emb"], np.float32)
    att_w1 = np.asarray(inputs["att_w1"], np.float32)
    att_w2 = np.asarray(inputs["att_w2"], np.float32)
    att_w3 = np.asarray(inputs["att_w3"], np.float32)
    gate_w1 = np.asarray(inputs["gate_w1"], np.float32)
    gate_w2 = np.asarray(inputs["gate_w2"], np.float32)
    items = np.asarray(inputs["items"]).astype(np.int64)
    idx6 = {n: np.asarray(inputs[n]).astype(np.int64)
            for n in ("user_h", "user_r", "user_t", "item_h", "item_r",
                      "item_t")}

    entp = np.zeros((N_ENTITY, 2 * DIM), np.float16)
    entp[:, 0:DIM] = ent.astype(np.float16)
    w1h = np.concatenate([gate_w1[:DIM], att_w1[:DIM]], axis=1)
    r1p = rel @ np.concatenate([gate_w1[DIM:], att_w1[DIM:]], axis=1)
    w1r = np.concatenate([w1h, r1p], axis=0).astype(np.float16)
    wflip = np.zeros((128, 65), np.float16)
    wflip[0:64, 0:64] = gate_w2.astype(np.float16)
    wflip[64:128, 64:65] = att_w3.astype(np.float16)

    in_maps = []
    for c in range(N_CORES):
        sl = slice(c * b_core, (c + 1) * b_core)
        im = {
            "entp": entp, "w1r": w1r, "w2": att_w2.astype(np.float16),
            "wflip": wflip,
            "items16": _wrap_idx16(items[sl].astype(np.int16)),
        }
        for k, (tw, l) in enumerate(TL_LIST):
            pre = "user" if tw == "u" else "item"
            h = idx6[f"{pre}_h"][l, sl].T.ravel()
            t = idx6[f"{pre}_t"][l, sl].T.ravel()
            r = idx6[f"{pre}_r"][l, sl].T.ravel()
            ia, hl, tl_ = _host_prep_tl(h, t)
            im[f"idxA{k}"] = _wrap_idx16(ia)
            # per-quarter [h_cols ++ t_cols] index stream
            cq = n_col // NQ
            parts = []
            for qu in range(NQ):
                parts.append(_wrap_idx16(hl[qu * cq:(qu + 1) * cq]))
                parts.append(_wrap_idx16(tl_[qu * cq:(qu + 1) * cq]))
            im[f"idxQ{k}"] = np.concatenate(parts, axis=1)
            oh = (r[None, :] == np.arange(N_RELATION)[:, None])
            im[f"ohr{k}"] = oh.astype(np.float16)
        in_maps.append(im)
    return in_maps


def kernel(**inputs):
    global _NC_CACHE
    import os
    from concourse.bass_utils import run_bass_kernel_spmd

    if _NC_CACHE is None:
        _NC_CACHE = _build_program()
    nc = _NC_CACHE
    in_maps = _make_in_maps(inputs)
    trace = bool(int(os.environ.get("CKAN_TRACE", "0")))
    res = run_bass_kernel_spmd(nc, in_maps, core_ids=list(range(N_CORES)),
                               trace=trace)
    if trace and res.exec_time_ns is not None:
        print(f"HW exec time: {res.exec_time_ns} ns")
    if trace and res.instructions_and_trace is not None:
        print(f"trace path: {res.instructions_and_trace[1]}")
    b_core = B // N_CORES
    out = np.concatenate([res.results[c]["scores"] for c in range(N_CORES)])
    return out.astype(np.float32)
